# revision 18
# baseline (speedup 1.0000x reference)
"""Multi-head attention (nn_AttentionMechanism) on 8 Trainium2 NeuronCores.

Reference computation (per batch n):
    v = values @ Wv.T ; k = keys @ Wk.T ; q = query @ Wq.T   (all [S, D])
    energy[h,i,j] = sum_d q[i,h,d] k[j,h,d]
    attn = softmax(energy / sqrt(D), axis=j)
    out = (attn @ v per head, concat heads) @ Wo.T + bo

Sharding: data-parallel over (batch, seq-half): core c handles batch c//2,
query rows (c%2)*1024..+1024. K/V are computed for the full 2048-row sequence
on both cores of a pair (duplicated compute, zero collectives).

On-chip strategy (per core):
 - Matmul operands in fp16 (1 cycle/row on the PE + fast weight load);
   accumulation in fp32 PSUM. Inputs are PE-transposed in fp32, cast to fp16
   on the PSUM->SBUF copy.
 - Projections q/k produce TRANSPOSED outputs (head-dim on partitions).
   Energy is computed transposed ([k-part, q-free]) so the softmax
   denominator rides along as a ones-column in the attn@v matmul.
 - k/v projections run per head-pair INSIDE the attention loop so their PE
   work overlaps the (bottleneck) ScalarE exp stream; Wo transposes are
   likewise spread across the attention pairs.
 - Heads processed in pairs: the two K=64 energy matmuls occupy different
   row-groups of the PE array and run concurrently; their exps are fused
   into one 1024-wide ACTIVATE.
 - Softmax without max-subtraction (energy/32 is ~N(0, 0.25); exp never
   overflows for this problem's input distribution).
"""

import numpy as np

import concourse.bass as bass
import concourse.mybir as mybir
import concourse.tile as tile
from concourse import bacc
from concourse.bass_utils import run_bass_kernel_spmd

F32 = mybir.dt.float32
F16 = mybir.dt.float16
AF = mybir.ActivationFunctionType
ALU = mybir.AluOpType

P = 128
D = 1024
H = 16
DH = 64
NQ = 1024  # q rows per core
NK = 2048  # kv rows per core
SCALE = 1.0 / 32.0  # 1/sqrt(D)

_CACHE = {}


def build():
    nc = bacc.Bacc("TRN2", target_bir_lowering=False, debug=False)

    xq = nc.dram_tensor("xq", [NQ, D], F32, kind="ExternalInput")
    xk = nc.dram_tensor("xk", [NK, D], F32, kind="ExternalInput")
    xv = nc.dram_tensor("xv", [NK, D], F32, kind="ExternalInput")
    wq = nc.dram_tensor("wq", [D, D], F32, kind="ExternalInput")
    wk = nc.dram_tensor("wk", [D, D], F32, kind="ExternalInput")
    wv = nc.dram_tensor("wv", [D, D], F32, kind="ExternalInput")
    wo = nc.dram_tensor("wo", [D, D], F32, kind="ExternalInput")
    bo = nc.dram_tensor("bo", [1, D], F32, kind="ExternalInput")
    ident_d = nc.dram_tensor("ident", [P, P], F32, kind="ExternalInput")
    ones_d = nc.dram_tensor("ones", [P, 2 * H], F16, kind="ExternalInput")
    out = nc.dram_tensor("out", [NQ, D], F32, kind="ExternalOutput")

    with tile.TileContext(nc) as tc:
        with (
            tc.tile_pool(name="consts", bufs=1) as consts,
            tc.tile_pool(name="glob", bufs=1) as glob,
        ):
            ident = consts.tile([P, P], F32)
            nc.sync.dma_start(ident[:], ident_d[:])
            ident16 = consts.tile([P, P], F16, name="ident16")
            nc.vector.tensor_copy(ident16[:], ident[:])

            qT = glob.tile([P, 8, NQ], F16, name="qT")      # 16 KB/part
            catT = glob.tile([P, 8, NQ], F16, name="catT")  # 16 KB/part

            with (
                tc.tile_pool(name="bglob", bufs=1) as bglob,
                tc.tile_pool(name="wtp", bufs=1) as wtp,
            ):
                xkT = bglob.tile([P, 8, NK], F16, name="xkT")   # 32 KB
                xvT = bglob.tile([P, 8, NK], F16, name="xvT")   # 32 KB
                wkT = wtp.tile([P, 8, D], F16, name="wkT")      # 16 KB
                wvT = wtp.tile([P, 8, D], F16, name="wvT")      # 16 KB

                # ---------------- Phase A ----------------
                with (
                    tc.tile_pool(name="xin", bufs=2) as xinp,
                    tc.tile_pool(name="wta", bufs=1) as wtap,
                    tc.tile_pool(name="psA", bufs=1, space="PSUM") as psA,
                ):
                    # alternate engines on the psum->sbuf cast copies
                    _eng = [0]

                    def _copy(dst, src):
                        if _eng[0] % 2 == 0:
                            nc.vector.tensor_copy(dst, src)
                        else:
                            nc.scalar.copy(dst, src)
                        _eng[0] += 1

                    def build_wT(w_dram, wT, wname):
                        """wT[:, dc, oc*128:+128] = W[oc-chunk, dc-chunk].T"""
                        for oc in range(8):
                            wnat = xinp.tile(
                                [P, D], F32, tag="xnat", name=f"{wname}_n{oc}"
                            )
                            nc.sync.dma_start(
                                wnat[:], w_dram[oc * P : (oc + 1) * P, :]
                            )
                            for dcq in range(2):
                                ps = psA.tile(
                                    [P, 512], F32, tag="tps", bufs=4,
                                    name=f"{wname}_t{oc}_{dcq}",
                                )
                                for j in range(4):
                                    nc.tensor.transpose(
                                        ps[:, j * P : (j + 1) * P],
                                        wnat[
                                            :,
                                            (dcq * 4 + j) * P : (dcq * 4 + j + 1) * P,
                                        ],
                                        ident[:],
                                    )
                                _copy(
                                    wT[
                                        :,
                                        dcq * 4 : (dcq + 1) * 4,
                                        oc * P : (oc + 1) * P,
                                    ],
                                    ps[:].rearrange("p (j c) -> p j c", c=P),
                                )

                    def build_xT(x_dram, xT, ib, xname):
                        """xT[:, dc, ib*512 ...] = x[i-block ib].T (fp16)"""
                        xnat = xinp.tile(
                            [P, 4, D], F32, tag="xbig", name=f"{xname}_n{ib}"
                        )
                        nc.sync.dma_start(
                            xnat[:],
                            x_dram[ib * 512 : (ib + 1) * 512, :].rearrange(
                                "(s p) d -> p s d", p=P
                            ),
                        )
                        for dc in range(8):
                            ps = psA.tile(
                                [P, 512], F32, tag="tps", bufs=4,
                                name=f"{xname}_t{ib}_{dc}",
                            )
                            for s in range(4):
                                nc.tensor.transpose(
                                    ps[:, s * P : (s + 1) * P],
                                    xnat[:, s, dc * P : (dc + 1) * P],
                                    ident[:],
                                )
                            _copy(xT[:, dc, ib * 512 : (ib + 1) * 512], ps[:])

                    # q projection (all of it) + k/v input transposes
                    wqT = wtap.tile([P, 8, D], F16, tag="wt", name="wqT")
                    build_wT(wq, wqT, "wqT")
                    xqT = wtap.tile([P, 8, NQ], F16, tag="xqT", name="xqT")
                    for ib in range(2):
                        build_xT(xq, xqT, ib, "xqT")
                    for ib in range(2):
                        for oc in range(8):
                            qps = psA.tile(
                                [P, 512], F32, tag="qps", bufs=2,
                                name=f"qps{ib}_{oc}",
                            )
                            for dc in range(8):
                                nc.tensor.matmul(
                                    qps[:],
                                    wqT[:, dc, oc * P : (oc + 1) * P],
                                    xqT[:, dc, ib * 512 : (ib + 1) * 512],
                                    start=(dc == 0),
                                    stop=(dc == 7),
                                )
                            nc.scalar.copy(
                                qT[:, oc, ib * 512 : (ib + 1) * 512], qps[:]
                            )

                    build_wT(wk, wkT, "wkT")
                    build_wT(wv, wvT, "wvT")
                    for ib in range(4):
                        build_xT(xk, xkT, ib, "xkT")
                    for ib in range(4):
                        build_xT(xv, xvT, ib, "xvT")

                # woT lives from here (reuses phase-A space) through phase C
                with tc.tile_pool(name="wop", bufs=1) as wop:
                    woT = wop.tile([P, 8, D], F16, name="woT")  # 16 KB

                    # ---------------- Phase B ----------------
                    with (
                        tc.tile_pool(name="kv", bufs=2) as kvp,
                        tc.tile_pool(name="pp", bufs=4) as ppp,
                        tc.tile_pool(name="dd", bufs=3) as ddp,
                        tc.tile_pool(name="psB", bufs=1, space="PSUM") as psB,
                    ):

                        def make_preamble(c):
                            """Allocate pair-c tiles; return (kT, vx, steps).

                            Each step is a thunk emitting one chunk of the
                            k/v projection (plus Wo transposes) so it can be
                            interleaved into the previous pair's attention.
                            """
                            kT = kvp.tile([P, NK], F16, tag="kt", name=f"kT{c}")
                            vT = kvp.tile([P, NK], F16, tag="vt", name=f"vT{c}")
                            vx = kvp.tile(
                                [P, 16, 2, 65], F16, tag="vx", name=f"vx{c}"
                            )
                            steps = []

                            def ones_step():
                                nc.sync.dma_start(
                                    vx[:, :, :, 64:65],
                                    ones_d[:, :, None].rearrange(
                                        "p (kc t) u -> p kc t u", t=2
                                    ),
                                )

                            steps.append(ones_step)

                            def proj_step(wT, xT, dst, ic4, nm):
                                cell = {}

                                def _h1():
                                    cell["ps"] = psB.tile(
                                        [P, 512], F32, tag="kvps", bufs=2,
                                        name=f"{nm}{c}_{ic4}",
                                    )
                                    for dc in range(4):
                                        nc.tensor.matmul(
                                            cell["ps"][:],
                                            wT[:, dc, c * P : (c + 1) * P],
                                            xT[:, dc, ic4 * 512 : (ic4 + 1) * 512],
                                            start=(dc == 0),
                                            stop=False,
                                        )

                                def _h2():
                                    ps_ = cell["ps"]
                                    for dc in range(4, 8):
                                        nc.tensor.matmul(
                                            ps_[:],
                                            wT[:, dc, c * P : (c + 1) * P],
                                            xT[:, dc, ic4 * 512 : (ic4 + 1) * 512],
                                            start=False,
                                            stop=(dc == 7),
                                        )
                                    nc.vector.tensor_copy(
                                        dst[:, ic4 * 512 : (ic4 + 1) * 512],
                                        ps_[:],
                                    )

                                return [_h1, _h2]

                            for ic4 in range(4):
                                steps.extend(proj_step(wkT, xkT, kT, ic4, "kps"))
                            for ic4 in range(4):
                                steps.extend(proj_step(wvT, xvT, vT, ic4, "vps"))

                            def vt_step(kc16):
                                def _f():
                                    for k2 in (kc16, kc16 + 1):
                                        tvp = psB.tile(
                                            [P, P], F16, tag="kvps", bufs=2,
                                            name=f"tvp{c}_{k2}",
                                        )
                                        nc.tensor.transpose(
                                            tvp[:],
                                            vT[:, k2 * P : (k2 + 1) * P],
                                            ident16[:],
                                        )
                                        nc.vector.tensor_copy(
                                            vx[:, k2, :, 0:64],
                                            tvp[:].rearrange(
                                                "p (t c) -> p t c", c=64
                                            ),
                                        )

                                return _f

                            for kc16 in range(0, 16, 2):
                                steps.append(vt_step(kc16))

                            # spread the Wo transpose-build over pairs 2..5
                            if 2 <= c <= 5:
                                def wo_step(oc):
                                    def _f():
                                        wnat = kvp.tile(
                                            [P, D], F32, tag="vt",
                                            name=f"woT_n{oc}",
                                        )
                                        nc.sync.dma_start(
                                            wnat[:],
                                            wo[oc * P : (oc + 1) * P, :],
                                        )
                                        for dcq in range(2):
                                            pw = psB.tile(
                                                [P, 512], F32, tag="kvps",
                                                bufs=2,
                                                name=f"woT_t{oc}_{dcq}",
                                            )
                                            for j in range(4):
                                                nc.tensor.transpose(
                                                    pw[:, j * P : (j + 1) * P],
                                                    wnat[
                                                        :,
                                                        (dcq * 4 + j) * P : (dcq * 4 + j + 1) * P,
                                                    ],
                                                    ident[:],
                                                )
                                            nc.vector.tensor_copy(
                                                woT[
                                                    :,
                                                    dcq * 4 : (dcq + 1) * 4,
                                                    oc * P : (oc + 1) * P,
                                                ],
                                                pw[:].rearrange(
                                                    "p (j c) -> p j c", c=P
                                                ),
                                            )

                                    return _f

                                for oc in (2 * (c - 2), 2 * (c - 2) + 1):
                                    steps.append(wo_step(oc))

                            return kT, vx, steps

                        # prologue: pair 0's projections run un-overlapped
                        kT, vx, steps = make_preamble(0)
                        for st in steps:
                            st()

                        for c in range(8):  # head pair
                            if c < 7:
                                kT_n, vx_n, steps = make_preamble(c + 1)
                            else:
                                kT_n, vx_n, steps = None, None, []
                            si = 0
                            for qt in range(2):
                                o0 = psB.tile(
                                    [65, 512], F32, tag="o0", bufs=1,
                                    name=f"o0_{c}_{qt}",
                                )
                                o1 = psB.tile(
                                    [65, 512], F32, tag="o1", bufs=1,
                                    name=f"o1_{c}_{qt}",
                                )
                                def energy(kc):
                                    ee = psB.tile(
                                        [P, 1024], F32, tag="ee", bufs=2,
                                        name=f"ee_{c}_{qt}_{kc}",
                                    )
                                    nc.tensor.matmul(
                                        ee[:, 0:512],
                                        kT[0:DH, kc * P : (kc + 1) * P],
                                        qT[0:DH, c, qt * 512 : (qt + 1) * 512],
                                        start=True,
                                        stop=True,
                                    )
                                    nc.tensor.matmul(
                                        ee[:, 512:1024],
                                        kT[DH:P, kc * P : (kc + 1) * P],
                                        qT[DH:P, c, qt * 512 : (qt + 1) * 512],
                                        start=True,
                                        stop=True,
                                    )
                                    pp = ppp.tile(
                                        [P, 1024], F16, tag="pp",
                                        name=f"pp_{c}_{qt}_{kc}",
                                    )
                                    nc.scalar.activation(
                                        pp[:], ee[:], AF.Exp, scale=SCALE
                                    )
                                    return pp

                                # energy runs one iteration ahead of attn@v
                                # so the in-order PE stream never stalls on
                                # the exp of the current iteration.
                                pp_cur = energy(0)
                                for kc in range(16):
                                    if kc < 15:
                                        pp_nxt = energy(kc + 1)
                                    nc.tensor.matmul(
                                        o0[:],
                                        vx[:, kc, 0, :],
                                        pp_cur[:, 0:512],
                                        start=(kc == 0),
                                        stop=(kc == 15),
                                    )
                                    nc.tensor.matmul(
                                        o1[:],
                                        vx[:, kc, 1, :],
                                        pp_cur[:, 512:1024],
                                        start=(kc == 0),
                                        stop=(kc == 15),
                                    )
                                    if kc < 15:
                                        pp_cur = pp_nxt
                                    # interleave one next-pair preamble step
                                    if si < len(steps):
                                        steps[si]()
                                        si += 1
                                # normalize: catT[rows, c, qt] = o[0:64]/o[64]
                                for j, ops in enumerate((o0, o1)):
                                    stage = ddp.tile(
                                        [P, 512], F32, tag="stage",
                                        name=f"stage{c}_{qt}_{j}",
                                    )
                                    nc.vector.tensor_copy(
                                        stage[0:65, :], ops[0:65, :]
                                    )
                                    dsh = ddp.tile(
                                        [1, 512], F32, tag="dsh",
                                        name=f"dsh{c}_{qt}_{j}",
                                    )
                                    nc.sync.dma_start(
                                        dsh[0:1, :], stage[64:65, :]
                                    )
                                    rec = ddp.tile(
                                        [P, 512], F32, tag="rec",
                                        name=f"rec{c}_{qt}_{j}",
                                    )
                                    nc.vector.reciprocal_approx_fast(
                                        out=rec[0:1, :], in_=dsh[0:1, :]
                                    )
                                    bc = ddp.tile(
                                        [DH, 512], F32, tag="bc",
                                        name=f"bc{c}_{qt}_{j}",
                                    )
                                    nc.gpsimd.partition_broadcast(
                                        bc[:], rec[0:1, :]
                                    )
                                    if j == 0:
                                        nc.vector.tensor_tensor(
                                            catT[
                                                0:DH, c, qt * 512 : (qt + 1) * 512
                                            ],
                                            stage[0:DH, :],
                                            bc[:],
                                            ALU.mult,
                                        )
                                    else:
                                        stg = ddp.tile(
                                            [DH, 512], F16, tag="stg",
                                            name=f"stg{c}_{qt}",
                                        )
                                        nc.vector.tensor_tensor(
                                            stg[:], stage[0:DH, :], bc[:],
                                            ALU.mult,
                                        )
                                        nc.sync.dma_start(
                                            catT[
                                                DH:P, c, qt * 512 : (qt + 1) * 512
                                            ],
                                            stg[:],
                                        )
                            # any remaining preamble steps
                            while si < len(steps):
                                steps[si]()
                                si += 1
                            kT, vx = kT_n, vx_n

                    # ---------------- Phase C: output projection ----------
                    with (
                        tc.tile_pool(name="osb", bufs=3) as osbp,
                        tc.tile_pool(name="psC", bufs=1, space="PSUM") as psC,
                    ):
                        bo_st = osbp.tile([P, D], F32, tag="bo_st", name="bo_st")
                        nc.sync.dma_start(bo_st[0:1, :], bo[:])
                        bo_bc = osbp.tile([P, D], F32, tag="bo_bc", name="bo_bc")
                        nc.gpsimd.partition_broadcast(bo_bc[:], bo_st[0:1, :])

                        for ic in range(8):
                            ot = osbp.tile([P, D], F32, tag="ot", name=f"ot{ic}")
                            for oc2 in range(2):
                                ops_ = psC.tile(
                                    [P, 512], F32, tag="ops", bufs=2,
                                    name=f"ops{ic}_{oc2}",
                                )
                                for dc in range(8):
                                    nc.tensor.matmul(
                                        ops_[:],
                                        catT[:, dc, ic * P : (ic + 1) * P],
                                        woT[:, dc, oc2 * 512 : (oc2 + 1) * 512],
                                        start=(dc == 0),
                                        stop=(dc == 7),
                                    )
                                nc.vector.tensor_tensor(
                                    ot[:, oc2 * 512 : (oc2 + 1) * 512],
                                    ops_[:],
                                    bo_bc[:, oc2 * 512 : (oc2 + 1) * 512],
                                    ALU.add,
                                )
                            nc.sync.dma_start(out[ic * P : (ic + 1) * P, :], ot[:])

    nc.compile()
    return nc


def _get_nc():
    if "nc" not in _CACHE:
        _CACHE["nc"] = build()
    return _CACHE["nc"]


def build_in_maps(inputs):
    values = np.ascontiguousarray(inputs["values"], dtype=np.float32)
    keys = np.ascontiguousarray(inputs["keys"], dtype=np.float32)
    query = np.ascontiguousarray(inputs["query"], dtype=np.float32)
    Wv = np.ascontiguousarray(inputs["Wv"], dtype=np.float32)
    Wk = np.ascontiguousarray(inputs["Wk"], dtype=np.float32)
    Wq = np.ascontiguousarray(inputs["Wq"], dtype=np.float32)
    Wo = np.ascontiguousarray(inputs["Wo"], dtype=np.float32)
    bo_ = np.ascontiguousarray(inputs["bo"], dtype=np.float32).reshape(1, D)
    ident = np.eye(P, dtype=np.float32)
    ones = np.ones((P, 2 * H), dtype=np.float16)
    in_maps = []
    for c in range(8):
        b, half = c // 2, c % 2
        in_maps.append(
            {
                "xq": np.ascontiguousarray(
                    query[b, half * NQ : (half + 1) * NQ, :]
                ),
                "xk": keys[b],
                "xv": values[b],
                "wq": Wq,
                "wk": Wk,
                "wv": Wv,
                "wo": Wo,
                "bo": bo_,
                "ident": ident,
                "ones": ones,
            }
        )
    return in_maps


def kernel(values, keys, query, Wv, Wk, Wq, Wo, bo):
    inputs = {
        "values": values, "keys": keys, "query": query,
        "Wv": Wv, "Wk": Wk, "Wq": Wq, "Wo": Wo, "bo": bo,
    }
    in_maps = build_in_maps(inputs)
    nc = _get_nc()
    res = run_bass_kernel_spmd(nc, in_maps, core_ids=list(range(8)))

    B, S = 4, 2048
    out = np.empty((B, S, D), dtype=np.float32)
    for c in range(8):
        b, half = c // 2, c % 2
        out[b, half * NQ : (half + 1) * NQ, :] = res.results[c]["out"]
    return out


# revision 19
# speedup vs baseline: 1.0447x; 1.0447x over previous
"""Multi-head attention (nn_AttentionMechanism) on 8 Trainium2 NeuronCores.

Reference computation (per batch n):
    v = values @ Wv.T ; k = keys @ Wk.T ; q = query @ Wq.T   (all [S, D])
    energy[h,i,j] = sum_d q[i,h,d] k[j,h,d]
    attn = softmax(energy / sqrt(D), axis=j)
    out = (attn @ v per head, concat heads) @ Wo.T + bo

Sharding: data-parallel over (batch, seq-half): core c handles batch c//2,
query rows (c%2)*1024..+1024. K/V are computed for the full 2048-row sequence
on both cores of a pair (duplicated compute, zero collectives).

On-chip strategy (per core):
 - Matmul operands in fp16 (1 cycle/row on the PE + fast weight load);
   accumulation in fp32 PSUM. Inputs are PE-transposed in fp32, cast to fp16
   on the PSUM->SBUF copy.
 - Projections q/k produce TRANSPOSED outputs (head-dim on partitions).
   Energy is computed transposed ([k-part, q-free]) so the softmax
   denominator rides along as a ones-column in the attn@v matmul.
 - k/v projections run per head-pair INSIDE the attention loop so their PE
   work overlaps the (bottleneck) ScalarE exp stream; Wo transposes are
   likewise spread across the attention pairs.
 - Heads processed in pairs: the two K=64 energy matmuls occupy different
   row-groups of the PE array and run concurrently; their exps are fused
   into one 1024-wide ACTIVATE.
 - Softmax without max-subtraction (energy/32 is ~N(0, 0.25); exp never
   overflows for this problem's input distribution).
"""

import numpy as np

import concourse.bass as bass
import concourse.mybir as mybir
import concourse.tile as tile
from concourse import bacc
from concourse.bass_utils import run_bass_kernel_spmd

F32 = mybir.dt.float32
F16 = mybir.dt.float16
AF = mybir.ActivationFunctionType
ALU = mybir.AluOpType

P = 128
D = 1024
H = 16
DH = 64
NQ = 1024  # q rows per core
NK = 2048  # kv rows per core
SCALE = 1.0 / 32.0  # 1/sqrt(D)

_CACHE = {}


def build():
    nc = bacc.Bacc("TRN2", target_bir_lowering=False, debug=False)

    xq = nc.dram_tensor("xq", [NQ, D], F32, kind="ExternalInput")
    xk = nc.dram_tensor("xk", [NK, D], F32, kind="ExternalInput")
    xv = nc.dram_tensor("xv", [NK, D], F32, kind="ExternalInput")
    wq = nc.dram_tensor("wq", [D, D], F32, kind="ExternalInput")
    wk = nc.dram_tensor("wk", [D, D], F32, kind="ExternalInput")
    wv = nc.dram_tensor("wv", [D, D], F32, kind="ExternalInput")
    wo = nc.dram_tensor("wo", [D, D], F32, kind="ExternalInput")
    bo = nc.dram_tensor("bo", [1, D], F32, kind="ExternalInput")
    ident_d = nc.dram_tensor("ident", [P, P], F32, kind="ExternalInput")
    ones_d = nc.dram_tensor("ones", [P, 2 * H], F16, kind="ExternalInput")
    out = nc.dram_tensor("out", [NQ, D], F32, kind="ExternalOutput")

    with tile.TileContext(nc) as tc:
        with (
            tc.tile_pool(name="consts", bufs=1) as consts,
            tc.tile_pool(name="glob", bufs=1) as glob,
        ):
            ident = consts.tile([P, P], F32)
            nc.sync.dma_start(ident[:], ident_d[:])
            ident16 = consts.tile([P, P], F16, name="ident16")
            nc.vector.tensor_copy(ident16[:], ident[:])

            qT = glob.tile([P, 8, NQ], F16, name="qT")      # 16 KB/part
            catT = glob.tile([P, 8, NQ], F16, name="catT")  # 16 KB/part

            with (
                tc.tile_pool(name="bglob", bufs=1) as bglob,
                tc.tile_pool(name="wtp", bufs=1) as wtp,
            ):
                xkT = bglob.tile([P, 8, NK], F16, name="xkT")   # 32 KB
                xvT = bglob.tile([P, 8, NK], F16, name="xvT")   # 32 KB
                wkT = wtp.tile([P, 8, D], F16, name="wkT")      # 16 KB
                wvT = wtp.tile([P, 8, D], F16, name="wvT")      # 16 KB

                # ---------------- Phase A ----------------
                with (
                    tc.tile_pool(name="xin", bufs=2) as xinp,
                    tc.tile_pool(name="wta", bufs=1) as wtap,
                    tc.tile_pool(name="psA", bufs=1, space="PSUM") as psA,
                ):
                    # alternate engines on the psum->sbuf cast copies
                    _eng = [0]

                    def _copy(dst, src):
                        if _eng[0] % 2 == 0:
                            nc.vector.tensor_copy(dst, src)
                        else:
                            nc.scalar.copy(dst, src)
                        _eng[0] += 1

                    def build_wT(w_dram, wT, wname):
                        """wT[:, dc, oc*128:+128] = W[oc-chunk, dc-chunk].T"""
                        for oc in range(8):
                            wnat = xinp.tile(
                                [P, D], F32, tag="xnat", name=f"{wname}_n{oc}"
                            )
                            nc.sync.dma_start(
                                wnat[:], w_dram[oc * P : (oc + 1) * P, :]
                            )
                            for dcq in range(2):
                                ps = psA.tile(
                                    [P, 512], F32, tag="tps", bufs=4,
                                    name=f"{wname}_t{oc}_{dcq}",
                                )
                                for j in range(4):
                                    nc.tensor.transpose(
                                        ps[:, j * P : (j + 1) * P],
                                        wnat[
                                            :,
                                            (dcq * 4 + j) * P : (dcq * 4 + j + 1) * P,
                                        ],
                                        ident[:],
                                    )
                                _copy(
                                    wT[
                                        :,
                                        dcq * 4 : (dcq + 1) * 4,
                                        oc * P : (oc + 1) * P,
                                    ],
                                    ps[:].rearrange("p (j c) -> p j c", c=P),
                                )

                    def build_xT(x_dram, xT, ib, xname):
                        """xT[:, dc, ib*512 ...] = x[i-block ib].T (fp16)"""
                        xnat = xinp.tile(
                            [P, 4, D], F32, tag="xbig", name=f"{xname}_n{ib}"
                        )
                        nc.sync.dma_start(
                            xnat[:],
                            x_dram[ib * 512 : (ib + 1) * 512, :].rearrange(
                                "(s p) d -> p s d", p=P
                            ),
                        )
                        for dc in range(8):
                            ps = psA.tile(
                                [P, 512], F32, tag="tps", bufs=4,
                                name=f"{xname}_t{ib}_{dc}",
                            )
                            for s in range(4):
                                nc.tensor.transpose(
                                    ps[:, s * P : (s + 1) * P],
                                    xnat[:, s, dc * P : (dc + 1) * P],
                                    ident[:],
                                )
                            _copy(xT[:, dc, ib * 512 : (ib + 1) * 512], ps[:])

                    # q projection (all of it) + k/v input transposes
                    wqT = wtap.tile([P, 8, D], F16, tag="wt", name="wqT")
                    build_wT(wq, wqT, "wqT")
                    xqT = wtap.tile([P, 8, NQ], F16, tag="xqT", name="xqT")
                    for ib in range(2):
                        build_xT(xq, xqT, ib, "xqT")
                    for ib in range(2):
                        for oc in range(8):
                            qps = psA.tile(
                                [P, 512], F32, tag="qps", bufs=2,
                                name=f"qps{ib}_{oc}",
                            )
                            for dc in range(8):
                                nc.tensor.matmul(
                                    qps[:],
                                    wqT[:, dc, oc * P : (oc + 1) * P],
                                    xqT[:, dc, ib * 512 : (ib + 1) * 512],
                                    start=(dc == 0),
                                    stop=(dc == 7),
                                )
                            nc.scalar.copy(
                                qT[:, oc, ib * 512 : (ib + 1) * 512], qps[:]
                            )

                    build_wT(wk, wkT, "wkT")
                    build_wT(wv, wvT, "wvT")
                    for ib in range(4):
                        build_xT(xk, xkT, ib, "xkT")
                    for ib in range(4):
                        build_xT(xv, xvT, ib, "xvT")

                # woT lives from here (reuses phase-A space) through phase C
                with tc.tile_pool(name="wop", bufs=1) as wop:
                    woT = wop.tile([P, 8, D], F16, name="woT")  # 16 KB

                    # ---------------- Phase B ----------------
                    with (
                        tc.tile_pool(name="kv", bufs=2) as kvp,
                        tc.tile_pool(name="pp", bufs=4) as ppp,
                        tc.tile_pool(name="dd", bufs=3) as ddp,
                        tc.tile_pool(name="psB", bufs=1, space="PSUM") as psB,
                    ):

                        def make_preamble(c):
                            """Allocate pair-c tiles; return (kT, vx, steps).

                            Each step is a thunk emitting one chunk of the
                            k/v projection (plus Wo transposes) so it can be
                            interleaved into the previous pair's attention.
                            """
                            kT = kvp.tile([P, NK], F16, tag="kt", name=f"kT{c}")
                            vT = kvp.tile([P, NK], F16, tag="vt", name=f"vT{c}")
                            vx = kvp.tile(
                                [P, 16, 2, 65], F16, tag="vx", name=f"vx{c}"
                            )
                            steps = []

                            def ones_step():
                                nc.sync.dma_start(
                                    vx[:, :, :, 64:65],
                                    ones_d[:, :, None].rearrange(
                                        "p (kc t) u -> p kc t u", t=2
                                    ),
                                )

                            steps.append(ones_step)

                            def proj_step(wT, xT, dst, ic4, nm):
                                def _f():
                                    ps_ = psB.tile(
                                        [P, 512], F32, tag="kvps", bufs=2,
                                        name=f"{nm}{c}_{ic4}",
                                    )
                                    for dc in range(8):
                                        nc.tensor.matmul(
                                            ps_[:],
                                            wT[:, dc, c * P : (c + 1) * P],
                                            xT[:, dc, ic4 * 512 : (ic4 + 1) * 512],
                                            start=(dc == 0),
                                            stop=(dc == 7),
                                        )
                                    nc.vector.tensor_copy(
                                        dst[:, ic4 * 512 : (ic4 + 1) * 512],
                                        ps_[:],
                                    )

                                return _f

                            for ic4 in range(4):
                                steps.append(proj_step(wkT, xkT, kT, ic4, "kps"))
                            for ic4 in range(4):
                                steps.append(proj_step(wvT, xvT, vT, ic4, "vps"))

                            def vt_step(kc16):
                                def _f():
                                    for k2 in (kc16, kc16 + 1):
                                        tvp = psB.tile(
                                            [P, P], F16, tag="kvps", bufs=2,
                                            name=f"tvp{c}_{k2}",
                                        )
                                        nc.tensor.transpose(
                                            tvp[:],
                                            vT[:, k2 * P : (k2 + 1) * P],
                                            ident16[:],
                                        )
                                        nc.vector.tensor_copy(
                                            vx[:, k2, :, 0:64],
                                            tvp[:].rearrange(
                                                "p (t c) -> p t c", c=64
                                            ),
                                        )

                                return _f

                            for kc16 in range(0, 16, 2):
                                steps.append(vt_step(kc16))

                            # spread the Wo transpose-build over pairs 2..5
                            if 2 <= c <= 5:
                                def wo_step(oc):
                                    def _f():
                                        wnat = kvp.tile(
                                            [P, D], F32, tag="vt",
                                            name=f"woT_n{oc}",
                                        )
                                        nc.sync.dma_start(
                                            wnat[:],
                                            wo[oc * P : (oc + 1) * P, :],
                                        )
                                        for dcq in range(2):
                                            pw = psB.tile(
                                                [P, 512], F32, tag="kvps",
                                                bufs=2,
                                                name=f"woT_t{oc}_{dcq}",
                                            )
                                            for j in range(4):
                                                nc.tensor.transpose(
                                                    pw[:, j * P : (j + 1) * P],
                                                    wnat[
                                                        :,
                                                        (dcq * 4 + j) * P : (dcq * 4 + j + 1) * P,
                                                    ],
                                                    ident[:],
                                                )
                                            nc.vector.tensor_copy(
                                                woT[
                                                    :,
                                                    dcq * 4 : (dcq + 1) * 4,
                                                    oc * P : (oc + 1) * P,
                                                ],
                                                pw[:].rearrange(
                                                    "p (j c) -> p j c", c=P
                                                ),
                                            )

                                    return _f

                                for oc in (2 * (c - 2), 2 * (c - 2) + 1):
                                    steps.append(wo_step(oc))

                            return kT, vx, steps

                        # prologue: pair 0's projections run un-overlapped
                        kT, vx, steps = make_preamble(0)
                        for st in steps:
                            st()

                        for c in range(8):  # head pair
                            if c < 7:
                                kT_n, vx_n, steps = make_preamble(c + 1)
                            else:
                                kT_n, vx_n, steps = None, None, []
                            si = 0
                            for qt in range(2):
                                o0 = psB.tile(
                                    [65, 512], F32, tag="o0", bufs=1,
                                    name=f"o0_{c}_{qt}",
                                )
                                o1 = psB.tile(
                                    [65, 512], F32, tag="o1", bufs=1,
                                    name=f"o1_{c}_{qt}",
                                )
                                def energy(kc):
                                    ee = psB.tile(
                                        [P, 1024], F32, tag="ee", bufs=2,
                                        name=f"ee_{c}_{qt}_{kc}",
                                    )
                                    nc.tensor.matmul(
                                        ee[:, 0:512],
                                        kT[0:DH, kc * P : (kc + 1) * P],
                                        qT[0:DH, c, qt * 512 : (qt + 1) * 512],
                                        start=True,
                                        stop=True,
                                    )
                                    nc.tensor.matmul(
                                        ee[:, 512:1024],
                                        kT[DH:P, kc * P : (kc + 1) * P],
                                        qT[DH:P, c, qt * 512 : (qt + 1) * 512],
                                        start=True,
                                        stop=True,
                                    )
                                    pp = ppp.tile(
                                        [P, 1024], F16, tag="pp",
                                        name=f"pp_{c}_{qt}_{kc}",
                                    )
                                    nc.scalar.activation(
                                        pp[:], ee[:], AF.Exp, scale=SCALE
                                    )
                                    return pp

                                # energy runs one iteration ahead of attn@v
                                # so the in-order PE stream never stalls on
                                # the exp of the current iteration.
                                pp_cur = energy(0)
                                for kc in range(16):
                                    if kc < 15:
                                        pp_nxt = energy(kc + 1)
                                    nc.tensor.matmul(
                                        o0[:],
                                        vx[:, kc, 0, :],
                                        pp_cur[:, 0:512],
                                        start=(kc == 0),
                                        stop=(kc == 15),
                                    )
                                    nc.tensor.matmul(
                                        o1[:],
                                        vx[:, kc, 1, :],
                                        pp_cur[:, 512:1024],
                                        start=(kc == 0),
                                        stop=(kc == 15),
                                    )
                                    if kc < 15:
                                        pp_cur = pp_nxt
                                    # interleave one next-pair preamble step
                                    # every other iteration
                                    if kc % 2 == 1 and si < len(steps):
                                        steps[si]()
                                        si += 1
                                # normalize: catT[rows, c, qt] = o[0:64]/o[64]
                                for j, ops in enumerate((o0, o1)):
                                    stage = ddp.tile(
                                        [P, 512], F32, tag="stage",
                                        name=f"stage{c}_{qt}_{j}",
                                    )
                                    nc.vector.tensor_copy(
                                        stage[0:65, :], ops[0:65, :]
                                    )
                                    dsh = ddp.tile(
                                        [1, 512], F32, tag="dsh",
                                        name=f"dsh{c}_{qt}_{j}",
                                    )
                                    nc.sync.dma_start(
                                        dsh[0:1, :], stage[64:65, :]
                                    )
                                    rec = ddp.tile(
                                        [P, 512], F32, tag="rec",
                                        name=f"rec{c}_{qt}_{j}",
                                    )
                                    nc.vector.reciprocal_approx_fast(
                                        out=rec[0:1, :], in_=dsh[0:1, :]
                                    )
                                    bc = ddp.tile(
                                        [DH, 512], F32, tag="bc",
                                        name=f"bc{c}_{qt}_{j}",
                                    )
                                    nc.gpsimd.partition_broadcast(
                                        bc[:], rec[0:1, :]
                                    )
                                    if j == 0:
                                        nc.vector.tensor_tensor(
                                            catT[
                                                0:DH, c, qt * 512 : (qt + 1) * 512
                                            ],
                                            stage[0:DH, :],
                                            bc[:],
                                            ALU.mult,
                                        )
                                    else:
                                        stg = ddp.tile(
                                            [DH, 512], F16, tag="stg",
                                            name=f"stg{c}_{qt}",
                                        )
                                        nc.vector.tensor_tensor(
                                            stg[:], stage[0:DH, :], bc[:],
                                            ALU.mult,
                                        )
                                        nc.sync.dma_start(
                                            catT[
                                                DH:P, c, qt * 512 : (qt + 1) * 512
                                            ],
                                            stg[:],
                                        )
                            # any remaining preamble steps
                            while si < len(steps):
                                steps[si]()
                                si += 1
                            kT, vx = kT_n, vx_n

                    # ---------------- Phase C: output projection ----------
                    with (
                        tc.tile_pool(name="osb", bufs=3) as osbp,
                        tc.tile_pool(name="psC", bufs=1, space="PSUM") as psC,
                    ):
                        bo_st = osbp.tile([P, D], F32, tag="bo_st", name="bo_st")
                        nc.sync.dma_start(bo_st[0:1, :], bo[:])
                        bo_bc = osbp.tile([P, D], F32, tag="bo_bc", name="bo_bc")
                        nc.gpsimd.partition_broadcast(bo_bc[:], bo_st[0:1, :])

                        for ic in range(8):
                            ot = osbp.tile([P, D], F32, tag="ot", name=f"ot{ic}")
                            for oc2 in range(2):
                                ops_ = psC.tile(
                                    [P, 512], F32, tag="ops", bufs=2,
                                    name=f"ops{ic}_{oc2}",
                                )
                                for dc in range(8):
                                    nc.tensor.matmul(
                                        ops_[:],
                                        catT[:, dc, ic * P : (ic + 1) * P],
                                        woT[:, dc, oc2 * 512 : (oc2 + 1) * 512],
                                        start=(dc == 0),
                                        stop=(dc == 7),
                                    )
                                nc.vector.tensor_tensor(
                                    ot[:, oc2 * 512 : (oc2 + 1) * 512],
                                    ops_[:],
                                    bo_bc[:, oc2 * 512 : (oc2 + 1) * 512],
                                    ALU.add,
                                )
                            nc.sync.dma_start(out[ic * P : (ic + 1) * P, :], ot[:])

    nc.compile()
    return nc


def _get_nc():
    if "nc" not in _CACHE:
        _CACHE["nc"] = build()
    return _CACHE["nc"]


def build_in_maps(inputs):
    values = np.ascontiguousarray(inputs["values"], dtype=np.float32)
    keys = np.ascontiguousarray(inputs["keys"], dtype=np.float32)
    query = np.ascontiguousarray(inputs["query"], dtype=np.float32)
    Wv = np.ascontiguousarray(inputs["Wv"], dtype=np.float32)
    Wk = np.ascontiguousarray(inputs["Wk"], dtype=np.float32)
    Wq = np.ascontiguousarray(inputs["Wq"], dtype=np.float32)
    Wo = np.ascontiguousarray(inputs["Wo"], dtype=np.float32)
    bo_ = np.ascontiguousarray(inputs["bo"], dtype=np.float32).reshape(1, D)
    ident = np.eye(P, dtype=np.float32)
    ones = np.ones((P, 2 * H), dtype=np.float16)
    in_maps = []
    for c in range(8):
        b, half = c // 2, c % 2
        in_maps.append(
            {
                "xq": np.ascontiguousarray(
                    query[b, half * NQ : (half + 1) * NQ, :]
                ),
                "xk": keys[b],
                "xv": values[b],
                "wq": Wq,
                "wk": Wk,
                "wv": Wv,
                "wo": Wo,
                "bo": bo_,
                "ident": ident,
                "ones": ones,
            }
        )
    return in_maps


def kernel(values, keys, query, Wv, Wk, Wq, Wo, bo):
    inputs = {
        "values": values, "keys": keys, "query": query,
        "Wv": Wv, "Wk": Wk, "Wq": Wq, "Wo": Wo, "bo": bo,
    }
    in_maps = build_in_maps(inputs)
    nc = _get_nc()
    res = run_bass_kernel_spmd(nc, in_maps, core_ids=list(range(8)))

    B, S = 4, 2048
    out = np.empty((B, S, D), dtype=np.float32)
    for c in range(8):
        b, half = c // 2, c % 2
        out[b, half * NQ : (half + 1) * NQ, :] = res.results[c]["out"]
    return out
